# revision 10
# baseline (speedup 1.0000x reference)
"""Trainium2 Bass kernel for AttnDecoderRNN step (8-core SPMD).

Sharding:
  - emb_table column-sharded (E/8 per core) -> on-device row gather by token
  - linear contraction-sharded -> AllReduce of [1,H] partials
  - attention replicated (tiny)
  - comb / GRU output-sharded (H/8 slices per core) -> AllGather
  - out projection vocab-sharded (V/8 rows per core), log-softmax via
    per-core sum(exp) + AllGather of the 8 partial sums

All activation vectors live in "col-form" [128, C] SBUF tiles
(h = c*128 + p), which is both the natural PE-matmul operand shape and
the AllGather rank-concat order.
"""

import sys

if "/opt/trn_rl_repo" not in sys.path:
    sys.path.insert(0, "/opt/trn_rl_repo")

import numpy as np

import concourse.bass as bass
import concourse.bacc as bacc
import concourse.tile as tile
import concourse.mybir as mybir
from concourse.bass_utils import run_bass_kernel_spmd

F32 = mybir.dt.float32
I32 = mybir.dt.int32

V, E, H, L = 50257, 1024, 1024, 52
NC = 8
VSH = 6283           # per-core vocab shard; 8*6283 = 50264 >= V
NCOL = 50            # 50*128 = 6400 >= 6283 padded shard
VPAD = NCOL * 128
NEG = -1.0e4         # pad bias; exp underflows to exactly 0 in fp32

# out-projection streaming strips (in units of 128-column groups)
STRIP_COLS = [7, 7, 6, 6, 6, 6, 6, 6]
assert sum(STRIP_COLS) == NCOL
STRIP_BUFS = 3

RG = [list(range(NC))]

_compiled = {}


def _build(n_iters=1):
    nc = bacc.Bacc("TRN2", target_bir_lowering=False, debug=False, num_devices=NC)

    # ---- I/O declarations (per-core shards prepared on host) ----
    tok_d = nc.dram_tensor("tok", [1, 1], I32, kind="ExternalInput")
    embc_d = nc.dram_tensor("embc", [V, 128], F32, kind="ExternalInput")
    linwT_d = nc.dram_tensor("linwT", [128, H], F32, kind="ExternalInput")
    lbcol_d = nc.dram_tensor("lbcol", [128, 8], F32, kind="ExternalInput")
    attnwT_d = nc.dram_tensor("attnwT", [2 * H, L], F32, kind="ExternalInput")
    attnb_d = nc.dram_tensor("attnb", [L, 1], F32, kind="ExternalInput")
    enc_d = nc.dram_tensor("enc", [L, H], F32, kind="ExternalInput")
    combT_d = nc.dram_tensor("combT", [2 * H, 128], F32, kind="ExternalInput")
    cbk_d = nc.dram_tensor("cbk", [128, 1], F32, kind="ExternalInput")
    wihT_d = nc.dram_tensor("wihT", [H, 384], F32, kind="ExternalInput")
    whhT_d = nc.dram_tensor("whhT", [H, 384], F32, kind="ExternalInput")
    brz_d = nc.dram_tensor("brz", [128, 2], F32, kind="ExternalInput")
    bin_d = nc.dram_tensor("bin", [128, 1], F32, kind="ExternalInput")
    bhn_d = nc.dram_tensor("bhn", [128, 1], F32, kind="ExternalInput")
    h0col_d = nc.dram_tensor("h0col", [128, 8], F32, kind="ExternalInput")
    h0k_d = nc.dram_tensor("h0k", [128, 1], F32, kind="ExternalInput")
    vwT_d = nc.dram_tensor("vwT", [H, VPAD], F32, kind="ExternalInput")
    obrow_d = nc.dram_tensor("obrow", [1, VPAD], F32, kind="ExternalInput")
    ident_d = nc.dram_tensor("ident", [128, 128], F32, kind="ExternalInput")
    onesc_d = nc.dram_tensor("onesc", [128, 1], F32, kind="ExternalInput")
    onesr_d = nc.dram_tensor("onesr", [1, 128], F32, kind="ExternalInput")

    lp_out = nc.dram_tensor("lp", [1, VPAD], F32, kind="ExternalOutput")
    hnew_out = nc.dram_tensor("hnew", [8, 128], F32, kind="ExternalOutput")
    attnw_out = nc.dram_tensor("attnw", [1, L], F32, kind="ExternalOutput")

    with tile.TileContext(nc) as tc:
      for _it in range(n_iters):
        with (
            tc.tile_pool(name="const", bufs=1) as cpool,
            tc.tile_pool(name="wts", bufs=1) as wpool,
            tc.tile_pool(name="act", bufs=1) as apool,
            tc.tile_pool(name="stage", bufs=2) as spool,
            tc.tile_pool(name="strip", bufs=STRIP_BUFS) as strip_pool,
            tc.tile_pool(name="psum", bufs=3, space="PSUM") as ppool,
            tc.tile_pool(name="psum1", bufs=1, space="PSUM") as ppool1,
            tc.tile_pool(name="dram", bufs=1, space="DRAM") as dpool,
        ):
            # ---- constants / small weights (issued first: HWDGE FIFO) ----
            ident = cpool.tile([128, 128], F32, tag="ident")
            nc.sync.dma_start(ident[:], ident_d[:])
            onesc = cpool.tile([128, 1], F32, tag="onesc")
            nc.sync.dma_start(onesc[:], onesc_d[:])
            onesr = cpool.tile([1, 128], F32, tag="onesr")
            nc.sync.dma_start(onesr[:], onesr_d[:])

            linwT = wpool.tile([128, H], F32, tag="linwT")
            nc.sync.dma_start(linwT[:], linwT_d[:])
            lbcol = wpool.tile([128, 8], F32, tag="lbcol")
            nc.sync.dma_start(lbcol[:], lbcol_d[:])
            h0col = wpool.tile([128, 8], F32, tag="h0col")
            nc.sync.dma_start(h0col[:], h0col_d[:])
            h0k = wpool.tile([128, 1], F32, tag="h0k")
            nc.sync.dma_start(h0k[:], h0k_d[:])
            attnwT = wpool.tile([128, 16, L], F32, tag="attnwT")
            nc.sync.dma_start(
                attnwT[:], attnwT_d[:].rearrange("(c p) f -> p c f", p=128)
            )
            attnb = wpool.tile([L, 1], F32, tag="attnb")
            nc.sync.dma_start(attnb[:], attnb_d[:])
            enc = wpool.tile([L, H], F32, tag="enc")
            nc.sync.dma_start(enc[:], enc_d[:])
            combT = wpool.tile([128, 16, 128], F32, tag="combT")
            nc.sync.dma_start(combT[:], combT_d[:].rearrange("(c p) f -> p c f", p=128))
            cbk = wpool.tile([128, 1], F32, tag="cbk")
            nc.sync.dma_start(cbk[:], cbk_d[:])
            wihT = wpool.tile([128, 8, 384], F32, tag="wihT")
            nc.sync.dma_start(wihT[:], wihT_d[:].rearrange("(c p) f -> p c f", p=128))
            whhT = wpool.tile([128, 8, 384], F32, tag="whhT")
            nc.sync.dma_start(whhT[:], whhT_d[:].rearrange("(c p) f -> p c f", p=128))
            brz = wpool.tile([128, 2], F32, tag="brz")
            nc.sync.dma_start(brz[:], brz_d[:])
            binn = wpool.tile([128, 1], F32, tag="binn")
            nc.sync.dma_start(binn[:], bin_d[:])
            bhn = wpool.tile([128, 1], F32, tag="bhn")
            nc.sync.dma_start(bhn[:], bhn_d[:])


            # ---- helpers ----
            def stage_to_dram(col_ap, c_sz, dram_tile, tag):
                """col-form [128, C] SBUF -> row-form [C, 128] DRAM."""
                tp = ppool.tile([c_sz, 128], F32, tag="seq")
                nc.tensor.transpose(tp[:], col_ap, ident[:])
                row = spool.tile([c_sz, 128], F32, tag=f"row_{tag}")
                nc.vector.tensor_copy(row[:], tp[:])
                nc.gpsimd.dma_start(dram_tile[:], row[:])
                return row

            def load_col(dram_tile, c_sz, tag, bias_ap=None):
                """row-form [C, 128] DRAM -> col-form [128, C] SBUF (+bias)."""
                row = spool.tile([c_sz, 128], F32, tag=f"lrow_{tag}")
                nc.gpsimd.dma_start(row[:], dram_tile[:])
                tp = ppool.tile([128, c_sz], F32, tag="seq")
                nc.tensor.transpose(tp[:], row[:], ident[0:c_sz, 0:c_sz])
                col = apool.tile([128, c_sz], F32, tag=f"lcol_{tag}")
                if bias_ap is None:
                    nc.vector.tensor_copy(col[:], tp[:])
                else:
                    nc.vector.tensor_tensor(
                        col[:], tp[:], bias_ap, mybir.AluOpType.add
                    )
                return col, row

            # ---- gh = h0 @ whh.T slices (no deps beyond weights: runs early) ----
            gh = ppool1.tile([128, 3], F32, tag="gh")
            for g in range(3):
                for c in range(8):
                    nc.tensor.matmul(
                        gh[:, g : g + 1],
                        whhT[:, c, g * 128 : (g + 1) * 128],
                        h0col[:, c : c + 1],
                        start=(c == 0),
                        stop=(c == 7),
                    )
            gh_sb = apool.tile([128, 3], F32, tag="gh_sb")
            nc.vector.tensor_copy(gh_sb[:], gh[:])

            # ---- embedding gather (dynamic row of [V, 128] shard) ----
            tokt = apool.tile([1, 1], I32, tag="tokt")
            nc.gpsimd.dma_start(tokt[:], tok_d[:])
            tok_val = nc.values_load(
                tokt[0:1, 0:1], min_val=0, max_val=V - 1,
                skip_runtime_bounds_check=True,
            )
            embrow = apool.tile([1, 128], F32, tag="embrow")
            nc.gpsimd.dma_start(embrow[:], embc_d[bass.ds(tok_val, 1), :])
            embp = ppool.tile([128, 1], F32, tag="seq")
            nc.tensor.transpose(embp[:], embrow[:], ident[0:1, 0:1])
            embcol = apool.tile([128, 1], F32, tag="embcol")
            nc.vector.tensor_copy(embcol[:], embp[:])

            # ---- linear partials: e1_part[:, hc] = linwT[:, hc-blk].T @ embcol
            e1p = ppool.tile([128, 8], F32, tag="seq")
            for hc in range(8):
                nc.tensor.matmul(
                    e1p[:, hc : hc + 1],
                    linwT[:, hc * 128 : (hc + 1) * 128],
                    embcol[:, 0:1],
                    start=True,
                    stop=True,
                )
            e1sb = apool.tile([128, 8], F32, tag="e1sb")
            nc.vector.tensor_copy(e1sb[:], e1p[:])

            cc1_in = dpool.tile([8, 128], F32, tag="cc1i")
            cc1_out = dpool.tile([8, 128], F32, tag="cc1o")
            stage_to_dram(e1sb[:], 8, cc1_in, "e1")
            nc.gpsimd.collective_compute(
                "AllReduce", mybir.AluOpType.add, replica_groups=RG,
                ins=[cc1_in.opt()], outs=[cc1_out.opt()],
            )
            e1col, _ = load_col(cc1_out, 8, "e1", bias_ap=lbcol[:])

            # ---- attention logits (col-form [52,1]) ----
            al = ppool.tile([L, 1], F32, tag="seq")
            for i, c in enumerate(list(range(8, 16)) + list(range(8))):
                rhs = e1col[:, c : c + 1] if c < 8 else h0col[:, c - 8 : c - 7]
                nc.tensor.matmul(
                    al[:, 0:1], attnwT[:, c, :], rhs, start=(i == 0), stop=(i == 15)
                )
            # softmax over 52 (inputs bounded -> no max subtraction needed)
            ex = apool.tile([L, 1], F32, tag="ex")
            nc.scalar.activation(
                ex[:], al[:], mybir.ActivationFunctionType.Exp, bias=attnb[:]
            )
            ssum = ppool.tile([1, 1], F32, tag="seq")
            nc.tensor.matmul(ssum[:], ex[:], onesc[0:L, 0:1], start=True, stop=True)
            ssb = apool.tile([1, 1], F32, tag="ssb")
            nc.vector.tensor_copy(ssb[:], ssum[:])
            rinv = apool.tile([1, 1], F32, tag="rinv")
            nc.vector.reciprocal(rinv[:], ssb[:])
            rinv52 = ppool.tile([L, 1], F32, tag="seq")
            nc.tensor.matmul(
                rinv52[:], onesr[0:1, 0:L], rinv[:], start=True, stop=True
            )
            aw = apool.tile([L, 1], F32, tag="aw")
            nc.vector.tensor_tensor(aw[:], ex[:], rinv52[:], mybir.AluOpType.mult)
            # attn_weights output (row form)
            awT = ppool.tile([1, L], F32, tag="seq")
            nc.tensor.transpose(awT[:], aw[:], ident[0:L, 0:L])
            awrow = apool.tile([1, L], F32, tag="awrow")
            nc.vector.tensor_copy(awrow[:], awT[:])
            nc.gpsimd.dma_start(attnw_out[:], awrow[:])

            # ---- attn_applied (col-form [128, 8]) ----
            aa = ppool.tile([128, 8], F32, tag="seq")
            for c in range(8):
                nc.tensor.matmul(
                    aa[:, c : c + 1],
                    enc[:, c * 128 : (c + 1) * 128],
                    aw[:, 0:1],
                    start=True,
                    stop=True,
                )
            aacol = apool.tile([128, 8], F32, tag="aacol")
            nc.vector.tensor_copy(aacol[:], aa[:])

            # ---- comb + relu -> x_k slice [128, 1] ----
            xk = ppool.tile([128, 1], F32, tag="seq")
            for c in range(16):
                rhs = e1col[:, c : c + 1] if c < 8 else aacol[:, c - 8 : c - 7]
                nc.tensor.matmul(
                    xk[:, 0:1], combT[:, c, :], rhs, start=(c == 0), stop=(c == 15)
                )
            xkcol = apool.tile([128, 1], F32, tag="xkcol")
            nc.scalar.activation(
                xkcol[:], xk[:], mybir.ActivationFunctionType.Relu, bias=cbk[:]
            )

            cc2_in = dpool.tile([1, 128], F32, tag="cc2i")
            cc2_out = dpool.tile([8, 128], F32, tag="cc2o")
            stage_to_dram(xkcol[:], 1, cc2_in, "x")
            nc.gpsimd.collective_compute(
                "AllGather", mybir.AluOpType.bypass, replica_groups=RG,
                ins=[cc2_in.opt()], outs=[cc2_out.opt()],
            )
            xcol, _ = load_col(cc2_out, 8, "x")

            # ---- gi = x @ wih.T slices ----
            gi = ppool1.tile([128, 3], F32, tag="gi")
            for g in range(3):
                for c in range(8):
                    nc.tensor.matmul(
                        gi[:, g : g + 1],
                        wihT[:, c, g * 128 : (g + 1) * 128],
                        xcol[:, c : c + 1],
                        start=(c == 0),
                        stop=(c == 7),
                    )

            # ---- gates ----
            trz = apool.tile([128, 2], F32, tag="trz")
            nc.vector.tensor_tensor(trz[:], gi[:, 0:2], gh_sb[:, 0:2], mybir.AluOpType.add)
            r_sb = apool.tile([128, 1], F32, tag="r_sb")
            nc.scalar.activation(
                r_sb[:], trz[:, 0:1], mybir.ActivationFunctionType.Sigmoid,
                bias=brz[:, 0:1],
            )
            z_sb = apool.tile([128, 1], F32, tag="z_sb")
            nc.scalar.activation(
                z_sb[:], trz[:, 1:2], mybir.ActivationFunctionType.Sigmoid,
                bias=brz[:, 1:2],
            )
            hn_sb = apool.tile([128, 1], F32, tag="hn_sb")
            nc.scalar.activation(
                hn_sb[:], gh_sb[:, 2:3], mybir.ActivationFunctionType.Identity,
                bias=bhn[:],
            )
            rhn = apool.tile([128, 1], F32, tag="rhn")
            nc.vector.tensor_tensor(rhn[:], r_sb[:], hn_sb[:], mybir.AluOpType.mult)
            tn = apool.tile([128, 1], F32, tag="tn")
            nc.vector.tensor_tensor(tn[:], gi[:, 2:3], rhn[:], mybir.AluOpType.add)
            n_sb = apool.tile([128, 1], F32, tag="n_sb")
            nc.scalar.activation(
                n_sb[:], tn[:], mybir.ActivationFunctionType.Tanh, bias=binn[:]
            )
            dmn = apool.tile([128, 1], F32, tag="dmn")
            nc.vector.tensor_tensor(dmn[:], h0k[:], n_sb[:], mybir.AluOpType.subtract)
            zd = apool.tile([128, 1], F32, tag="zd")
            nc.vector.tensor_tensor(zd[:], z_sb[:], dmn[:], mybir.AluOpType.mult)
            hnk = apool.tile([128, 1], F32, tag="hnk")
            nc.vector.tensor_tensor(hnk[:], n_sb[:], zd[:], mybir.AluOpType.add)

            cc3_in = dpool.tile([1, 128], F32, tag="cc3i")
            cc3_out = dpool.tile([8, 128], F32, tag="cc3o")
            stage_to_dram(hnk[:], 1, cc3_in, "hn")
            nc.gpsimd.collective_compute(
                "AllGather", mybir.AluOpType.bypass, replica_groups=RG,
                ins=[cc3_in.opt()], outs=[cc3_out.opt()],
            )
            hncol, hnrow = load_col(cc3_out, 8, "hn")
            nc.gpsimd.dma_start(hnew_out[:], hnrow[:])

            # ---- out projection: logits row-form, streamed per N-tile ----
            # lhsT = h_new chunk [128,1] (stationary, 1-col load), rhs = vwT
            # tile [128h, <=512v] (moving). Bias folded in via a K=1 matmul
            # against obrow. Each N-tile: 8 accum MMs + 1 bias MM -> psum
            # [1, <=512]; exp+accum and sbuf copy run per-tile, pipelined.
            lgrow = apool.tile([1, VPAD], F32, tag="lgrow")
            scol = apool.tile([1, 16], F32, tag="scol")
            nc.vector.memset(scol[:], 0.0)
            expscr = spool.tile([1, 512], F32, tag="expscr")
            t_idx = 0
            off = 0
            for s, w in enumerate(STRIP_COLS):
                tiles = []
                for c in range(8):
                    t = strip_pool.tile([128, w * 128], F32, tag=f"w{c}")
                    nc.sync.dma_start(
                        t[:, 0 : w * 128],
                        vwT_d[c * 128 : (c + 1) * 128, off * 128 : (off + w) * 128],
                    )
                    tiles.append(t)
                wel = w * 128
                for lo in range(0, wel, 512):
                    n_sz = min(512, wel - lo)
                    g0 = off * 128 + lo
                    lgr = ppool.tile([1, 512], F32, tag="lgr")
                    for c in range(8):
                        nc.tensor.matmul(
                            lgr[0:1, 0:n_sz],
                            hncol[:, c : c + 1],
                            tiles[c][:, lo : lo + n_sz],
                            start=(c == 0),
                            stop=False,
                        )
                    obch = spool.tile([1, 512], F32, tag="obch")
                    nc.sync.dma_start(obch[0:1, 0:n_sz], obrow_d[0:1, g0 : g0 + n_sz])
                    nc.tensor.matmul(
                        lgr[0:1, 0:n_sz],
                        onesc[0:1, 0:1],
                        obch[0:1, 0:n_sz],
                        start=False,
                        stop=True,
                    )
                    nc.scalar.activation(
                        expscr[0:1, 0:n_sz], lgr[0:1, 0:n_sz],
                        mybir.ActivationFunctionType.Exp,
                        accum_out=scol[0:1, t_idx : t_idx + 1],
                    )
                    nc.vector.tensor_copy(lgrow[0:1, g0 : g0 + n_sz], lgr[0:1, 0:n_sz])
                    t_idx += 1
                off += w

            # ---- log-softmax epilogue ----
            sloc = apool.tile([1, 1], F32, tag="sloc")
            nc.vector.tensor_reduce(
                sloc[:], scol[:], mybir.AxisListType.X, mybir.AluOpType.add
            )
            srow = spool.tile([1, 8], F32, tag="srow")
            nc.vector.memset(srow[:], 0.0)
            nc.scalar.copy(srow[0:1, 0:1], sloc[:])
            cc4_in = dpool.tile([1, 8], F32, tag="cc4i")
            cc4_out = dpool.tile([8, 8], F32, tag="cc4o")
            nc.gpsimd.dma_start(cc4_in[:], srow[:])
            nc.gpsimd.collective_compute(
                "AllGather", mybir.AluOpType.bypass, replica_groups=RG,
                ins=[cc4_in.opt()], outs=[cc4_out.opt()],
            )
            s8 = spool.tile([8, 8], F32, tag="s8")
            nc.gpsimd.dma_start(s8[:], cc4_out[:])
            stot = ppool.tile([1, 1], F32, tag="seq")
            nc.tensor.matmul(
                stot[:], s8[:, 0:1], onesc[0:8, 0:1], start=True, stop=True
            )
            logS = apool.tile([1, 1], F32, tag="logS")
            nc.scalar.activation(logS[:], stot[:], mybir.ActivationFunctionType.Ln)
            nlogS = apool.tile([1, 1], F32, tag="nlogS")
            nc.scalar.mul(nlogS[:], logS[:], -1.0)
            for lo in range(0, VPAD, 800):
                lpc = spool.tile([1, 800], F32, tag="lpc")
                nc.scalar.activation(
                    lpc[:], lgrow[0:1, lo : lo + 800],
                    mybir.ActivationFunctionType.Identity, bias=nlogS[:],
                )
                nc.sync.dma_start(lp_out[0:1, lo : lo + 800], lpc[:])

    nc.compile()
    return nc


def _prep_in_maps(token, hidden, encoder_outputs, emb_table, linear_w, linear_b,
                  attn_w, attn_b, comb_w, comb_b, gru_wih, gru_whh, gru_bih,
                  gru_bhh, out_w, out_b):
    f = np.float32
    tok = np.asarray(token).reshape(1, 1).astype(np.int32)
    h0 = np.asarray(hidden, f).reshape(H)
    h0col = np.ascontiguousarray(h0.reshape(8, 128).T)
    enc = np.ascontiguousarray(np.asarray(encoder_outputs, f))
    attnwT = np.ascontiguousarray(np.asarray(attn_w, f).T)
    attnb = np.asarray(attn_b, f).reshape(L, 1)
    lbcol = np.ascontiguousarray(np.asarray(linear_b, f).reshape(8, 128).T)

    bih = np.asarray(gru_bih, f)
    bhh = np.asarray(gru_bhh, f)

    ow_pad = np.zeros((NC * VSH, H), f)
    ow_pad[:V] = np.asarray(out_w, f)
    ob_pad = np.full(NC * VSH, NEG, f)
    ob_pad[:V] = np.asarray(out_b, f)

    ident = np.eye(128, dtype=f)
    onesc = np.ones((128, 1), f)
    onesr = np.ones((1, 128), f)

    emb = np.asarray(emb_table, f)
    lw = np.asarray(linear_w, f)
    cw = np.asarray(comb_w, f)
    wih = np.asarray(gru_wih, f)
    whh = np.asarray(gru_whh, f)

    in_maps = []
    for k in range(NC):
        ks = slice(k * 128, (k + 1) * 128)
        embc = np.ascontiguousarray(emb[:, ks])
        linwT = np.ascontiguousarray(lw[:, ks].T)
        combT = np.ascontiguousarray(cw[ks, :].T)
        wih_rows = np.concatenate([wih[g * H + k * 128:g * H + (k + 1) * 128]
                                   for g in range(3)], axis=0)
        whh_rows = np.concatenate([whh[g * H + k * 128:g * H + (k + 1) * 128]
                                   for g in range(3)], axis=0)
        wihT = np.ascontiguousarray(wih_rows.T)
        whhT = np.ascontiguousarray(whh_rows.T)
        brz = np.stack([(bih[0:H] + bhh[0:H])[ks],
                        (bih[H:2 * H] + bhh[H:2 * H])[ks]], axis=1)
        binn = bih[2 * H:3 * H][ks].reshape(128, 1)
        bhn = bhh[2 * H:3 * H][ks].reshape(128, 1)

        vs = slice(k * VSH, (k + 1) * VSH)
        vwT = np.zeros((H, VPAD), f)
        vwT[:, :VSH] = ow_pad[vs].T
        ob_sh = np.full(VPAD, NEG, f)
        ob_sh[:VSH] = ob_pad[vs]
        obrow = ob_sh.reshape(1, VPAD)

        in_maps.append({
            "tok": tok,
            "embc": embc,
            "linwT": linwT,
            "lbcol": lbcol,
            "attnwT": attnwT,
            "attnb": attnb,
            "enc": enc,
            "combT": combT,
            "cbk": np.asarray(comb_b, f)[ks].reshape(128, 1),
            "wihT": wihT,
            "whhT": whhT,
            "brz": np.ascontiguousarray(brz),
            "bin": binn,
            "bhn": bhn,
            "h0col": h0col,
            "h0k": h0[ks].reshape(128, 1).astype(f),
            "vwT": vwT,
            "obrow": obrow,
            "ident": ident,
            "onesc": onesc,
            "onesr": onesr,
        })
    return in_maps


def _get_nc(n_iters=1):
    if n_iters not in _compiled:
        _compiled[n_iters] = _build(n_iters)
    return _compiled[n_iters]


def run_on_device(in_maps, **kwargs):
    nc = _get_nc()
    return run_bass_kernel_spmd(nc, in_maps, list(range(NC)), **kwargs)


def kernel(**inputs):
    in_maps = _prep_in_maps(**inputs)
    res = run_on_device(in_maps).results

    shards = [res[k]["lp"].reshape(VPAD)[:VSH] for k in range(NC)]
    log_probs = np.concatenate(shards)[:V].reshape(1, V).astype(np.float32)
    h_new = res[0]["hnew"].reshape(1, 1, H).astype(np.float32)
    attn_weights = res[0]["attnw"].reshape(1, L).astype(np.float32)
    return (log_probs, h_new, attn_weights)
